# revision 45
# baseline (speedup 1.0000x reference)
"""Weighted-BCE loss kernel for Trainium2 (8 NeuronCores, SPMD data-parallel).

Reference math (torch-style BCELoss with class-balancing weights):
    n   = len(x), s = sum(gt)
    w0  = n / (2*(n-s)),  w1 = n / (2*s)
    L1  = max(log(x),     -100)
    L0  = max(log1p(-x),  -100)
    loss = mean( where(gt==0, w0, w1) * -(gt*L1 + (1-gt)*L0) )

Only ONE of log(x) / log(1-x) is needed per element (selected by gt), so
instead of two Ln passes we compute the selected operand in one shot:
    z = gt ? x : 1-x  =  1 - |x - gt|
The Ln pass uses bias 1 + 2^-24, so even the x==0, gt==1 corner (where
|w| == 1 exactly) stays finite: Ln sees 2^-24 -> -16.6.  vs the
reference's -100 clamp this misvalues only exact x==0 elements (~1 in
16.7M, error ~5e-6 of the loss); the +2^-24 shift itself biases the
mean by ~1e-6.  Global sums, all computed shard-locally:
    A  = sum(gt * Lz)   [DVE STT accum]   = sum_{gt=1} log x
    T  = sum(Lz)        [ACT accum, free on the Ln pass]
    s  = sum(x) - sum(w)  [PE colsum matmuls -> 2 PSUM banks, one DVE
                           psum-subtract at the end;  w = x - gt]
    loss = -( A/(2s) + (T-A)/(2(n-s)) )

Engine split (measured: DVE STT 1.08ns/col + ~0.15us/op, ACT pass
0.98ns/col, DMA 429 GB/s per core but only with >=16KB descriptors;
gpsimd tensor ops are unsupported/too slow, so the two 2-tensor ops
must both sit on DVE):
    DVE     w-STT (accum -> W) + A-STT (Lz, gt; accum -> A, deferred
            2 sub-tiles so it never head-blocks ready w-STTs in DVE's
            in-order queue while waiting on the cross-engine Ln)
    ACT     d = Abs(w) -> own tile; Lz = Ln(1+2^-23 - d) in place,
            accum -> T
    PE      colsum(x) 512 cols/matmul -> one PSUM bank (idle engine;
            same `ones` stationary throughout -> no reloads)
    SP      ALL input DMAs, pre-issued upfront as interleaved x/gt
            chunk pairs into two fully-RESIDENT SBUF tensors (64KB/
            partition each) - nothing downstream can stall the queue;
            ramping chunk sizes so compute starts ~11us.
A dummy Ln in the preamble pre-loads the natural_log act table
(abs/ln/copy) so no table swap lands mid-pipeline.  Host gathers the
[128, 2*NT] accums + the [1, 512] S-colsums from all 8 cores and
finishes the (tiny) all-reduce + scalar math in float64.
"""

import numpy as np
from contextlib import ExitStack

import bass_rust
import concourse.bass as bass
import concourse.bacc as bacc
import concourse.mybir as mybir
import concourse.tile as tile
from concourse.alu_op_type import AluOpType
from concourse.bass_utils import run_bass_kernel_spmd

N_TOTAL = 16777216
N_CORES = 8
PER_CORE = N_TOTAL // N_CORES   # 2097152
P = 128
FD = PER_CORE // P              # 16384 free elements per partition
# DMA chunk schedule, issued as interleaved x/gt pairs on one queue
CHUNKS = [512, 1024, 2048, 4096, 4096, 4096, 512]
assert sum(CHUNKS) == FD
# compute sub-tiles; each must lie inside a single DMA chunk
TILE_SIZES = [512, 1024, 2048, 2048, 2048, 2048, 2048, 2048, 1024, 512, 512, 512]
assert sum(TILE_SIZES) == FD
NT = len(TILE_SIZES)
MM = 512                        # moving free-dim chunk for PE colsums
LN_BIAS = 1.0 + 2.0**-23        # keeps Ln input >= 2^-23 even at |w| == 1
                                # (1 + 2^-24 would round to 1.0 in f32!)
LOG_CLAMP = -100.0

# Optional instrumentation knobs for a driver script (harness never sets them).
TRACE = False
LAST_RESULTS = None

_NC_CACHE = None


def _build():
    f32 = mybir.dt.float32
    i32 = mybir.dt.int32
    Ln = mybir.ActivationFunctionType.Ln
    Abs = mybir.ActivationFunctionType.Abs

    nc = bacc.Bacc("TRN2")
    x_in = nc.declare_dram_parameter("x", [P, FD], f32, isOutput=False)
    g_in = nc.declare_dram_parameter("gt", [P, FD], i32, isOutput=False)
    # packed accum output: columns [A | T | W], NT each
    out_all = nc.declare_dram_parameter("out_all", [P, 3 * NT], f32, isOutput=True)
    # column sums of x, summed on host; s = sum(x) - sum(W)
    sum_x = nc.declare_dram_parameter("sum_x", [1, MM], f32, isOutput=True)

    n_mm = FD // MM

    with tile.TileContext(nc) as tc, ExitStack() as ctx:
        resp = ctx.enter_context(tc.tile_pool(name="resp", bufs=1))
        wp = ctx.enter_context(tc.tile_pool(name="wp", bufs=3))
        dp = ctx.enter_context(tc.tile_pool(name="dp", bufs=4))
        jp = ctx.enter_context(tc.tile_pool(name="jp", bufs=2))
        accp = ctx.enter_context(tc.tile_pool(name="accp", bufs=1))
        pp = ctx.enter_context(tc.psum_pool(name="pp", bufs=1))

        # fully-resident input tensors
        x_sb = resp.tile([P, FD], f32)
        g_sb = resp.tile([P, FD], i32)

        # pre-issue every DMA on the single SP queue as x/gt pairs
        off = 0
        for cw in CHUNKS:
            cs, ce = off, off + cw
            off += cw
            nc.sync.dma_start(x_sb[:, cs:ce], x_in[:, cs:ce])
            nc.sync.dma_start(g_sb[:, cs:ce], g_in[:, cs:ce])

        # one packed accum tile -> one output DMA
        acc_all = accp.tile([P, 3 * NT], f32)

        ones = accp.tile([P, 1], f32)
        nc.gpsimd.memset(ones[:], 1.0)
        ln_bias = accp.tile([P, 1], f32)
        nc.vector.memset(ln_bias[:], LN_BIAS)

        # dummy Ln: forces the natural_log act-table (contains abs/ln/copy)
        # to load during the preamble instead of mid-pipeline
        warm = accp.tile([P, 1], f32)
        nc.scalar.activation(warm[:], ones[:], Ln)

        bank_x = pp.tile([1, MM], f32)

        def col(group, i):
            return acc_all[:, group * NT + i : group * NT + i + 1]

        def emit_A(i, lz, gsl, tfd):
            junk_a = jp.tile([P, tfd], f32, tag="junk_a")
            return nc.vector.scalar_tensor_tensor(
                junk_a[:], lz[:], LOG_CLAMP, g_sb[:, gsl],
                AluOpType.max, AluOpType.mult,
                accum_out=col(0, i),
            )

        pending_A = []  # (i, lz_tile, gt_slice, tfd): emitted 2 sub-tiles late
        mmx = 0
        off = 0
        for i, tfd in enumerate(TILE_SIZES):
            sl = slice(off, off + tfd)
            off += tfd

            # w = x - gt in (-1, 1]  (no clamp needed: Ln bias covers |w|=1)
            # accum -> W; s = sum(x) - sum(W) on the host
            wt = wp.tile([P, tfd], f32, tag="w")
            w_inst = nc.vector.scalar_tensor_tensor(
                wt[:], x_sb[:, sl], 0.0, g_sb[:, sl],
                AluOpType.max, AluOpType.subtract,
                accum_out=col(2, i),
            )
            # colsum(x) into one PSUM bank on the otherwise-idle PE
            for c in range(sl.start, sl.stop, MM):
                nc.tensor.matmul(
                    bank_x[:], ones[:], x_sb[:, c : c + MM],
                    start=(mmx == 0), stop=(mmx == n_mm - 1),
                )
                mmx += 1
            # ACT: d = |w|, then Lz = Ln(1 + 2^-24 - d) in place, accum -> T
            dt_ = dp.tile([P, tfd], f32, tag="d")
            nc.scalar.activation(dt_[:], wt[:], Abs)
            nc.scalar.activation(
                dt_[:], dt_[:], Ln, bias=ln_bias[:], scale=-1.0,
                accum_out=col(1, i),
            )
            # A-STT deferred two sub-tiles.  The Tile scheduler does NOT
            # respect emission order (it re-simulates with a cost model
            # whose DMA is slower than reality, so it pairs each A right
            # after its own w and DVE then stalls on Ln cross-engine
            # deps at real DMA speed).  Pin the intent with a no-sync
            # ordering edge: A_{i-2} goes after w_i in DVE's in-order
            # stream, by which time Ln_{i-2} is two sub-tiles old.
            pending_A.append((i, dt_, sl, tfd))
            if len(pending_A) > 2:
                a_inst = emit_A(*pending_A.pop(0))
                bass_rust.add_dep_helper(
                    a_inst.ins, w_inst.ins, sync=False,
                    reason="defer A two sub-tiles behind w",
                )

        # x colsums: PSUM -> SBUF -> DRAM
        sx_sb = accp.tile([1, MM], f32)
        nc.scalar.copy(sx_sb[:], bank_x[:])
        nc.sync.dma_start(sum_x[:, :], sx_sb[:])

        for args in pending_A:
            emit_A(*args)

        nc.sync.dma_start(out_all[:, :], acc_all[:])

    nc.compile()
    return nc


def get_nc():
    global _NC_CACHE
    if _NC_CACHE is None:
        _NC_CACHE = _build()
    return _NC_CACHE


def make_in_maps(x, gt):
    x = np.ascontiguousarray(np.asarray(x, dtype=np.float32).reshape(-1))
    gt = np.ascontiguousarray(np.asarray(gt, dtype=np.int32).reshape(-1))
    assert x.shape == (N_TOTAL,) and gt.shape == (N_TOTAL,)
    in_maps = []
    for c in range(N_CORES):
        sl = slice(c * PER_CORE, (c + 1) * PER_CORE)
        in_maps.append({
            "x": x[sl].reshape(P, FD),
            "gt": gt[sl].reshape(P, FD),
        })
    return in_maps


def combine(results):
    """All-reduce the per-core partial sums and finish the loss formula."""
    A = T = S = 0.0
    for r in results:
        o = r["out_all"].astype(np.float64)
        A += o[:, 0 * NT : 1 * NT].sum()
        T += o[:, 1 * NT : 2 * NT].sum()
        W = o[:, 2 * NT : 3 * NT].sum()
        S += r["sum_x"].astype(np.float64).sum() - W
    n = float(N_TOTAL)
    result = -(A / (2.0 * S) + (T - A) / (2.0 * (n - S)))
    return np.array(result, dtype=np.float32)


def kernel(x, gt):
    global LAST_RESULTS
    nc = get_nc()
    in_maps = make_in_maps(x, gt)
    br = run_bass_kernel_spmd(nc, in_maps, list(range(N_CORES)))
    LAST_RESULTS = br
    return combine(br.results)


# revision 46
# speedup vs baseline: 16.4683x; 16.4683x over previous
"""Weighted-BCE loss kernel for Trainium2 (8 NeuronCores, SPMD data-parallel).

Reference math (torch-style BCELoss with class-balancing weights):
    n   = len(x), s = sum(gt)
    w0  = n / (2*(n-s)),  w1 = n / (2*s)
    L1  = max(log(x),     -100)
    L0  = max(log1p(-x),  -100)
    loss = mean( where(gt==0, w0, w1) * -(gt*L1 + (1-gt)*L0) )

Only ONE of log(x) / log(1-x) is needed per element (selected by gt), so
instead of two Ln passes we compute the selected operand in one shot:
    z = gt ? x : 1-x  =  1 - |x - gt|
The Ln pass uses bias 1 + 2^-24, so even the x==0, gt==1 corner (where
|w| == 1 exactly) stays finite: Ln sees 2^-24 -> -16.6.  vs the
reference's -100 clamp this misvalues only exact x==0 elements (~1 in
16.7M, error ~5e-6 of the loss); the +2^-24 shift itself biases the
mean by ~1e-6.  Global sums, all computed shard-locally:
    A  = sum(gt * Lz)   [DVE STT accum]   = sum_{gt=1} log x
    T  = sum(Lz)        [ACT accum, free on the Ln pass]
    s  = sum(x) - sum(w)  [PE colsum matmuls -> 2 PSUM banks, one DVE
                           psum-subtract at the end;  w = x - gt]
    loss = -( A/(2s) + (T-A)/(2(n-s)) )

Engine split (measured: DVE STT 1.08ns/col + ~0.15us/op, ACT pass
0.98ns/col, DMA 429 GB/s per core but only with >=16KB descriptors;
gpsimd tensor ops are unsupported/too slow, so the two 2-tensor ops
must both sit on DVE):
    DVE     w-STT (accum -> W) + A-STT (Lz, gt; accum -> A, deferred
            2 sub-tiles so it never head-blocks ready w-STTs in DVE's
            in-order queue while waiting on the cross-engine Ln)
    ACT     d = Abs(w) -> own tile; Lz = Ln(1+2^-23 - d) in place,
            accum -> T
    PE      colsum(x) 512 cols/matmul -> one PSUM bank (idle engine;
            same `ones` stationary throughout -> no reloads)
    SP      ALL input DMAs, pre-issued upfront as interleaved x/gt
            chunk pairs into two fully-RESIDENT SBUF tensors (64KB/
            partition each) - nothing downstream can stall the queue;
            ramping chunk sizes so compute starts ~11us.
A dummy Ln in the preamble pre-loads the natural_log act table
(abs/ln/copy) so no table swap lands mid-pipeline.  Host gathers the
[128, 2*NT] accums + the [1, 512] S-colsums from all 8 cores and
finishes the (tiny) all-reduce + scalar math in float64.
"""

import numpy as np
from contextlib import ExitStack

import bass_rust
import concourse.bass as bass
import concourse.bacc as bacc
import concourse.mybir as mybir
import concourse.tile as tile
from concourse.alu_op_type import AluOpType
from concourse.bass_utils import run_bass_kernel_spmd

N_TOTAL = 16777216
N_CORES = 8
PER_CORE = N_TOTAL // N_CORES   # 2097152
P = 128
FD = PER_CORE // P              # 16384 free elements per partition
# DMA chunk schedule, issued as interleaved x/gt pairs on one queue
CHUNKS = [512, 1024, 2048, 2048, 2048, 4096, 4096, 512]
assert sum(CHUNKS) == FD
# compute sub-tiles; each must lie inside a single DMA chunk
TILE_SIZES = [512, 1024, 2048, 2048, 2048, 2048, 2048, 2048, 1024, 512, 512, 512]
assert sum(TILE_SIZES) == FD
NT = len(TILE_SIZES)
MM = 512                        # moving free-dim chunk for PE colsums
LN_BIAS = 1.0 + 2.0**-23        # keeps Ln input >= 2^-23 even at |w| == 1
                                # (1 + 2^-24 would round to 1.0 in f32!)
LOG_CLAMP = -100.0

# Optional instrumentation knobs for a driver script (harness never sets them).
TRACE = False
LAST_RESULTS = None

_NC_CACHE = None


def _build():
    f32 = mybir.dt.float32
    i32 = mybir.dt.int32
    Ln = mybir.ActivationFunctionType.Ln
    Abs = mybir.ActivationFunctionType.Abs

    nc = bacc.Bacc("TRN2")
    x_in = nc.declare_dram_parameter("x", [P, FD], f32, isOutput=False)
    g_in = nc.declare_dram_parameter("gt", [P, FD], i32, isOutput=False)
    # packed accum output: columns [A | T | W], NT each
    out_all = nc.declare_dram_parameter("out_all", [P, 3 * NT], f32, isOutput=True)
    # column sums of x, summed on host; s = sum(x) - sum(W)
    sum_x = nc.declare_dram_parameter("sum_x", [1, MM], f32, isOutput=True)

    n_mm = FD // MM

    with tile.TileContext(nc) as tc, ExitStack() as ctx:
        resp = ctx.enter_context(tc.tile_pool(name="resp", bufs=1))
        wp = ctx.enter_context(tc.tile_pool(name="wp", bufs=3))
        dp = ctx.enter_context(tc.tile_pool(name="dp", bufs=4))
        jp = ctx.enter_context(tc.tile_pool(name="jp", bufs=2))
        accp = ctx.enter_context(tc.tile_pool(name="accp", bufs=1))
        pp = ctx.enter_context(tc.psum_pool(name="pp", bufs=1))

        # fully-resident input tensors
        x_sb = resp.tile([P, FD], f32)
        g_sb = resp.tile([P, FD], i32)

        # pre-issue every DMA on the single SP queue as x/gt pairs
        off = 0
        for cw in CHUNKS:
            cs, ce = off, off + cw
            off += cw
            nc.sync.dma_start(x_sb[:, cs:ce], x_in[:, cs:ce])
            nc.sync.dma_start(g_sb[:, cs:ce], g_in[:, cs:ce])

        # one packed accum tile -> one output DMA
        acc_all = accp.tile([P, 3 * NT], f32)

        ones = accp.tile([P, 1], f32)
        nc.gpsimd.memset(ones[:], 1.0)
        ln_bias = accp.tile([P, 1], f32)
        nc.vector.memset(ln_bias[:], LN_BIAS)

        # dummy Ln: forces the natural_log act-table (contains abs/ln/copy)
        # to load during the preamble instead of mid-pipeline
        warm = accp.tile([P, 1], f32)
        nc.scalar.activation(warm[:], ones[:], Ln)

        bank_x = pp.tile([1, MM], f32)

        def col(group, i):
            return acc_all[:, group * NT + i : group * NT + i + 1]

        def emit_A(i, lz, gsl, tfd):
            junk_a = jp.tile([P, tfd], f32, tag="junk_a")
            return nc.vector.scalar_tensor_tensor(
                junk_a[:], lz[:], LOG_CLAMP, g_sb[:, gsl],
                AluOpType.max, AluOpType.mult,
                accum_out=col(0, i),
            )

        pending_A = []  # (i, lz_tile, gt_slice, tfd): emitted 2 sub-tiles late
        mmx = 0
        off = 0
        for i, tfd in enumerate(TILE_SIZES):
            sl = slice(off, off + tfd)
            off += tfd

            # w = x - gt in (-1, 1]  (no clamp needed: Ln bias covers |w|=1)
            # accum -> W; s = sum(x) - sum(W) on the host
            wt = wp.tile([P, tfd], f32, tag="w")
            w_inst = nc.vector.scalar_tensor_tensor(
                wt[:], x_sb[:, sl], 0.0, g_sb[:, sl],
                AluOpType.max, AluOpType.subtract,
                accum_out=col(2, i),
            )
            # colsum(x) into one PSUM bank on the otherwise-idle PE
            for c in range(sl.start, sl.stop, MM):
                nc.tensor.matmul(
                    bank_x[:], ones[:], x_sb[:, c : c + MM],
                    start=(mmx == 0), stop=(mmx == n_mm - 1),
                )
                mmx += 1
            # ACT: d = |w|, then Lz = Ln(1 + 2^-24 - d) in place, accum -> T
            dt_ = dp.tile([P, tfd], f32, tag="d")
            nc.scalar.activation(dt_[:], wt[:], Abs)
            nc.scalar.activation(
                dt_[:], dt_[:], Ln, bias=ln_bias[:], scale=-1.0,
                accum_out=col(1, i),
            )
            # A-STT deferred two sub-tiles.  The Tile scheduler does NOT
            # respect emission order (it re-simulates with a cost model
            # whose DMA is slower than reality, so it pairs each A right
            # after its own w and DVE then stalls on Ln cross-engine
            # deps at real DMA speed).  Pin the intent with a no-sync
            # ordering edge: A_{i-2} goes after w_i in DVE's in-order
            # stream, by which time Ln_{i-2} is two sub-tiles old.
            pending_A.append((i, dt_, sl, tfd))
            if len(pending_A) > 2:
                a_inst = emit_A(*pending_A.pop(0))
                bass_rust.add_dep_helper(
                    a_inst.ins, w_inst.ins, sync=False,
                    reason="defer A two sub-tiles behind w",
                )

        # x colsums: PSUM -> SBUF -> DRAM
        sx_sb = accp.tile([1, MM], f32)
        nc.scalar.copy(sx_sb[:], bank_x[:])
        nc.sync.dma_start(sum_x[:, :], sx_sb[:])

        for args in pending_A:
            emit_A(*args)

        nc.sync.dma_start(out_all[:, :], acc_all[:])

    nc.compile()
    return nc


def get_nc():
    global _NC_CACHE
    if _NC_CACHE is None:
        _NC_CACHE = _build()
    return _NC_CACHE


def make_in_maps(x, gt):
    x = np.ascontiguousarray(np.asarray(x, dtype=np.float32).reshape(-1))
    gt = np.ascontiguousarray(np.asarray(gt, dtype=np.int32).reshape(-1))
    assert x.shape == (N_TOTAL,) and gt.shape == (N_TOTAL,)
    in_maps = []
    for c in range(N_CORES):
        sl = slice(c * PER_CORE, (c + 1) * PER_CORE)
        in_maps.append({
            "x": x[sl].reshape(P, FD),
            "gt": gt[sl].reshape(P, FD),
        })
    return in_maps


def combine(results):
    """All-reduce the per-core partial sums and finish the loss formula."""
    A = T = S = 0.0
    for r in results:
        o = r["out_all"].astype(np.float64)
        A += o[:, 0 * NT : 1 * NT].sum()
        T += o[:, 1 * NT : 2 * NT].sum()
        W = o[:, 2 * NT : 3 * NT].sum()
        S += r["sum_x"].astype(np.float64).sum() - W
    n = float(N_TOTAL)
    result = -(A / (2.0 * S) + (T - A) / (2.0 * (n - S)))
    return np.array(result, dtype=np.float32)


def kernel(x, gt):
    global LAST_RESULTS
    nc = get_nc()
    in_maps = make_in_maps(x, gt)
    br = run_bass_kernel_spmd(nc, in_maps, list(range(N_CORES)))
    LAST_RESULTS = br
    return combine(br.results)


# revision 47
# speedup vs baseline: 18.4500x; 1.1203x over previous
"""Weighted-BCE loss kernel for Trainium2 (8 NeuronCores, SPMD data-parallel).

Reference math (torch-style BCELoss with class-balancing weights):
    n   = len(x), s = sum(gt)
    w0  = n / (2*(n-s)),  w1 = n / (2*s)
    L1  = max(log(x),     -100)
    L0  = max(log1p(-x),  -100)
    loss = mean( where(gt==0, w0, w1) * -(gt*L1 + (1-gt)*L0) )

Only ONE of log(x) / log(1-x) is needed per element (selected by gt), so
instead of two Ln passes we compute the selected operand in one shot:
    z = gt ? x : 1-x  =  1 - |x - gt|
The Ln pass uses bias 1 + 2^-24, so even the x==0, gt==1 corner (where
|w| == 1 exactly) stays finite: Ln sees 2^-24 -> -16.6.  vs the
reference's -100 clamp this misvalues only exact x==0 elements (~1 in
16.7M, error ~5e-6 of the loss); the +2^-24 shift itself biases the
mean by ~1e-6.  Global sums, all computed shard-locally:
    A  = sum(gt * Lz)   [DVE STT accum]   = sum_{gt=1} log x
    T  = sum(Lz)        [ACT accum, free on the Ln pass]
    s  = sum(x) - sum(w)  [PE colsum matmuls -> 2 PSUM banks, one DVE
                           psum-subtract at the end;  w = x - gt]
    loss = -( A/(2s) + (T-A)/(2(n-s)) )

Engine split (measured: DVE STT 1.08ns/col + ~0.15us/op, ACT pass
0.98ns/col, DMA 429 GB/s per core but only with >=16KB descriptors;
gpsimd tensor ops are unsupported/too slow, so the two 2-tensor ops
must both sit on DVE):
    DVE     w-STT (accum -> W) + A-STT (Lz, gt; accum -> A, deferred
            2 sub-tiles so it never head-blocks ready w-STTs in DVE's
            in-order queue while waiting on the cross-engine Ln)
    ACT     d = Abs(w) -> own tile; Lz = Ln(1+2^-23 - d) in place,
            accum -> T
    PE      colsum(x) 512 cols/matmul -> one PSUM bank (idle engine;
            same `ones` stationary throughout -> no reloads)
    SP      ALL input DMAs, pre-issued upfront as interleaved x/gt
            chunk pairs into two fully-RESIDENT SBUF tensors (64KB/
            partition each) - nothing downstream can stall the queue;
            ramping chunk sizes so compute starts ~11us.
A dummy Ln in the preamble pre-loads the natural_log act table
(abs/ln/copy) so no table swap lands mid-pipeline.  Host gathers the
[128, 2*NT] accums + the [1, 512] S-colsums from all 8 cores and
finishes the (tiny) all-reduce + scalar math in float64.
"""

import numpy as np
from contextlib import ExitStack

import bass_rust
import concourse.bass as bass
import concourse.bacc as bacc
import concourse.mybir as mybir
import concourse.tile as tile
from concourse.alu_op_type import AluOpType
from concourse.bass_utils import run_bass_kernel_spmd

N_TOTAL = 16777216
N_CORES = 8
PER_CORE = N_TOTAL // N_CORES   # 2097152
P = 128
FD = PER_CORE // P              # 16384 free elements per partition
# DMA chunk schedule, issued as interleaved x/gt pairs on one queue
# 4096-col chunks (16KB descriptors) are mandatory for peak DMA rate:
# splitting them 2048-wide was measured to LOSE more bandwidth than the
# finer completion granularity gains back
CHUNKS = [512, 1024, 2048, 4096, 4096, 4096, 512]
assert sum(CHUNKS) == FD
# compute sub-tiles; each must lie inside a single DMA chunk
TILE_SIZES = [512, 1024, 2048, 2048, 2048, 2048, 2048, 2048, 1024, 512, 512, 512]
assert sum(TILE_SIZES) == FD
NT = len(TILE_SIZES)
MM = 512                        # moving free-dim chunk for PE colsums
LN_BIAS = 1.0 + 2.0**-23        # keeps Ln input >= 2^-23 even at |w| == 1
                                # (1 + 2^-24 would round to 1.0 in f32!)
LOG_CLAMP = -100.0

# Optional instrumentation knobs for a driver script (harness never sets them).
TRACE = False
LAST_RESULTS = None

_NC_CACHE = None


def _build():
    f32 = mybir.dt.float32
    i32 = mybir.dt.int32
    Ln = mybir.ActivationFunctionType.Ln
    Abs = mybir.ActivationFunctionType.Abs

    nc = bacc.Bacc("TRN2")
    x_in = nc.declare_dram_parameter("x", [P, FD], f32, isOutput=False)
    g_in = nc.declare_dram_parameter("gt", [P, FD], i32, isOutput=False)
    # packed accum output: columns [A | T | W], NT each
    out_all = nc.declare_dram_parameter("out_all", [P, 3 * NT], f32, isOutput=True)
    # column sums of x, summed on host; s = sum(x) - sum(W)
    sum_x = nc.declare_dram_parameter("sum_x", [1, MM], f32, isOutput=True)

    n_mm = FD // MM

    with tile.TileContext(nc) as tc, ExitStack() as ctx:
        resp = ctx.enter_context(tc.tile_pool(name="resp", bufs=1))
        wp = ctx.enter_context(tc.tile_pool(name="wp", bufs=3))
        dp = ctx.enter_context(tc.tile_pool(name="dp", bufs=4))
        jp = ctx.enter_context(tc.tile_pool(name="jp", bufs=2))
        accp = ctx.enter_context(tc.tile_pool(name="accp", bufs=1))
        pp = ctx.enter_context(tc.psum_pool(name="pp", bufs=1))

        # fully-resident input tensors
        x_sb = resp.tile([P, FD], f32)
        g_sb = resp.tile([P, FD], i32)

        # pre-issue every DMA on the single SP queue as x/gt pairs
        off = 0
        for cw in CHUNKS:
            cs, ce = off, off + cw
            off += cw
            nc.sync.dma_start(x_sb[:, cs:ce], x_in[:, cs:ce])
            nc.sync.dma_start(g_sb[:, cs:ce], g_in[:, cs:ce])

        # one packed accum tile -> one output DMA
        acc_all = accp.tile([P, 3 * NT], f32)

        ones = accp.tile([P, 1], f32)
        nc.gpsimd.memset(ones[:], 1.0)
        ln_bias = accp.tile([P, 1], f32)
        nc.vector.memset(ln_bias[:], LN_BIAS)

        # dummy Ln: forces the natural_log act-table (contains abs/ln/copy)
        # to load during the preamble instead of mid-pipeline
        warm = accp.tile([P, 1], f32)
        nc.scalar.activation(warm[:], ones[:], Ln)

        bank_x = pp.tile([1, MM], f32)

        def col(group, i):
            return acc_all[:, group * NT + i : group * NT + i + 1]

        def emit_A(i, lz, gsl, tfd):
            junk_a = jp.tile([P, tfd], f32, tag="junk_a")
            return nc.vector.scalar_tensor_tensor(
                junk_a[:], lz[:], LOG_CLAMP, g_sb[:, gsl],
                AluOpType.max, AluOpType.mult,
                accum_out=col(0, i),
            )

        pending_A = []  # (i, lz_tile, gt_slice, tfd): emitted 2 sub-tiles late
        mmx = 0
        off = 0
        for i, tfd in enumerate(TILE_SIZES):
            sl = slice(off, off + tfd)
            off += tfd

            # w = x - gt in (-1, 1]  (no clamp needed: Ln bias covers |w|=1)
            # accum -> W; s = sum(x) - sum(W) on the host
            wt = wp.tile([P, tfd], f32, tag="w")
            w_inst = nc.vector.scalar_tensor_tensor(
                wt[:], x_sb[:, sl], 0.0, g_sb[:, sl],
                AluOpType.max, AluOpType.subtract,
                accum_out=col(2, i),
            )
            # colsum(x) into one PSUM bank on the otherwise-idle PE
            for c in range(sl.start, sl.stop, MM):
                nc.tensor.matmul(
                    bank_x[:], ones[:], x_sb[:, c : c + MM],
                    start=(mmx == 0), stop=(mmx == n_mm - 1),
                )
                mmx += 1
            # ACT: d = |w|, then Lz = Ln(1 + 2^-24 - d) in place, accum -> T
            dt_ = dp.tile([P, tfd], f32, tag="d")
            nc.scalar.activation(dt_[:], wt[:], Abs)
            nc.scalar.activation(
                dt_[:], dt_[:], Ln, bias=ln_bias[:], scale=-1.0,
                accum_out=col(1, i),
            )
            # A-STT deferred two sub-tiles.  The Tile scheduler does NOT
            # respect emission order (it re-simulates with a cost model
            # whose DMA is slower than reality, so it pairs each A right
            # after its own w and DVE then stalls on Ln cross-engine
            # deps at real DMA speed).  Pin the intent with a no-sync
            # ordering edge: A_{i-2} goes after w_i in DVE's in-order
            # stream, by which time Ln_{i-2} is two sub-tiles old.
            pending_A.append((i, dt_, sl, tfd))
            if len(pending_A) > 2:
                a_inst = emit_A(*pending_A.pop(0))
                bass_rust.add_dep_helper(
                    a_inst.ins, w_inst.ins, sync=False,
                    reason="defer A two sub-tiles behind w",
                )

        # x colsums: PSUM -> SBUF -> DRAM
        sx_sb = accp.tile([1, MM], f32)
        nc.scalar.copy(sx_sb[:], bank_x[:])
        nc.sync.dma_start(sum_x[:, :], sx_sb[:])

        for args in pending_A:
            emit_A(*args)

        nc.sync.dma_start(out_all[:, :], acc_all[:])

    nc.compile()
    return nc


def get_nc():
    global _NC_CACHE
    if _NC_CACHE is None:
        _NC_CACHE = _build()
    return _NC_CACHE


def make_in_maps(x, gt):
    x = np.ascontiguousarray(np.asarray(x, dtype=np.float32).reshape(-1))
    gt = np.ascontiguousarray(np.asarray(gt, dtype=np.int32).reshape(-1))
    assert x.shape == (N_TOTAL,) and gt.shape == (N_TOTAL,)
    in_maps = []
    for c in range(N_CORES):
        sl = slice(c * PER_CORE, (c + 1) * PER_CORE)
        in_maps.append({
            "x": x[sl].reshape(P, FD),
            "gt": gt[sl].reshape(P, FD),
        })
    return in_maps


def combine(results):
    """All-reduce the per-core partial sums and finish the loss formula."""
    A = T = S = 0.0
    for r in results:
        o = r["out_all"].astype(np.float64)
        A += o[:, 0 * NT : 1 * NT].sum()
        T += o[:, 1 * NT : 2 * NT].sum()
        W = o[:, 2 * NT : 3 * NT].sum()
        S += r["sum_x"].astype(np.float64).sum() - W
    n = float(N_TOTAL)
    result = -(A / (2.0 * S) + (T - A) / (2.0 * (n - S)))
    return np.array(result, dtype=np.float32)


def kernel(x, gt):
    global LAST_RESULTS
    nc = get_nc()
    in_maps = make_in_maps(x, gt)
    br = run_bass_kernel_spmd(nc, in_maps, list(range(N_CORES)))
    LAST_RESULTS = br
    return combine(br.results)
